# revision 2
# baseline (speedup 1.0000x reference)
"""Causal self-attention Bass/Tile kernel for Trainium2, 8-core data-parallel.

Problem: B=8, T=1024, C=1024, H=16, D=64, fp32.
  qkv = x @ w_attn + b_attn; causal SDPA over 16 heads; out = y @ w_proj + b_proj

Sharding: batch (B=8) across the 8 NeuronCores - one batch element per core.

v2 design:
  - x, w_attn loaded directly into f32r tiles (no conversion copies)
  - batched weight DMAs (1 per qk chunk / k-slice)
  - qkT, e, vaug, y, yT, w_proj in bf16 (1 cycle/row matmuls at any N)
  - AV computed output-natural: out[tq, d] via lhsT=e-slices, packed PSUM
    accumulators [128, 4, 65] (vaug ones-column gives softmax denominators
    per-partition); normalization via DVE broadcast-mult
  - y assembled + transposed via DMA-transpose (bf16) into yT for out-proj
  - biases folded: b_attn(q,k) added on qk psum drain; b_v @ w_proj + b_proj
    precomputed as b_eff, added via K=1 matmul into the out-proj psum
  - round-based emission interleaving qk-chunk matmuls with score/exp chains
"""

from contextlib import ExitStack

import numpy as np

import concourse.bass as bass
import concourse.tile as tile
from concourse import mybir
from concourse.bass_utils import run_bass_kernel_spmd
from concourse.masks import make_identity

F32 = mybir.dt.float32
F32R = mybir.dt.float32r
BF16 = mybir.dt.bfloat16
AF = mybir.ActivationFunctionType

# ---------------------------------------------------------------------------
# TRN2 allows at most 1 sem wait per instruction (2 on InstEventSemaphore).
# bacc's generate_event_semaphores pass legalizes multi-wait instructions.
# ---------------------------------------------------------------------------
import bass_rust as _bass_rust


def _strip_xpose_waits(nc):
    """walrus mishandles sem waits on InstDmaTransposeAnt; move them onto a
    NoOp just before the xpose in its engine stream."""
    uid = 0
    for f in nc.m.functions:
        for blk in f.blocks:
            insts = blk.instructions
            i = 0
            while i < len(insts):
                inst = insts[i]
                si = inst.sync_info
                if isinstance(inst, mybir.InstDmaTransposeAnt) and si is not None \
                        and len(si.on_wait) > 0:
                    waits = list(si.on_wait)
                    inst.sync_info = mybir.SyncInfo(
                        on_wait=[], on_update=list(si.on_update))
                    for w in waits:
                        nop = mybir.InstNoOp(
                            name=f"xwait-{uid}", engine=inst.engine, ins=[],
                            outs=[], sync_info=mybir.SyncInfo(on_wait=[w], on_update=[]),
                        )
                        uid += 1
                        insts.insert(i, nop)
                        i += 1
                i += 1


def _legalize_waits(nc):
    _strip_xpose_waits(nc)
    _bass_rust.generate_event_semaphores(nc)


# ---------------------------------------------------------------------------
N_CORES = 8
T = 1024
C = 1024
H = 16
D = C // H  # 64
C3 = 3 * C
P = 128
NT = T // P       # 8 t-chunks
NK = C // P       # 8 contraction chunks
TQG = 512         # tq group width
NG = T // TQG     # 2
HPAIRS = H // 2   # 8; hp covers heads 2hp (parts 0-63), 2hp+1 (parts 64-127)
SCALE = 1.0 / np.sqrt(D)


def _emit_kernel(nc, tc, ctx, x_d, wa_d, ba_d, wp_d, bp_d, out_d, dbg=None):
    def dump(name, ap_f32_src, shape):
        """Copy an sbuf/psum AP to f32 sbuf and DMA to the dbg dram tensor."""
        if dbg is None or name not in dbg:
            return
        with tc.tile_pool(name=f"dbg_{name}", bufs=1) as pool:
            t = pool.tile(shape, F32, name=f"dbg_{name}")
            nc.vector.tensor_copy(t.rearrange("p (a b) -> p a b", a=ap_f32_src.shape[1])
                                  if len(ap_f32_src.shape) == 3 else t, ap_f32_src)
            nc.sync.dma_start(out=dbg[name], in_=t)

    const = ctx.enter_context(tc.tile_pool(name="const", bufs=1))
    persist = ctx.enter_context(tc.tile_pool(name="persist", bufs=1))

    # --- constants / small loads ----------------------------------------
    ident_raw = const.tile([P, P], F32)
    make_identity(nc, ident_raw)
    ident = const.tile([P, P], F32R)
    nc.gpsimd.tensor_copy(ident, ident_raw)
    ones_bf = const.tile([1, P], BF16)
    nc.vector.memset(ones_bf, 1.0)
    ident_bf = const.tile([P, P], BF16)
    make_identity(nc, ident_bf)
    # b_attn q/k part as [128, 16] (partition p of column m = bias[m*128+p])
    ba_sb = const.tile([P, 2 * C // P], F32)
    nc.sync.dma_start(out=ba_sb, in_=ba_d[0 : 2 * C].rearrange("(m p) -> p m", p=P))
    # v-bias [128, 8] and proj-bias [1, 1024] as bf16 (for b_eff)
    bv_bf = const.tile([P, NK], BF16)
    nc.gpsimd.dma_start(out=bv_bf, in_=ba_d[2 * C : 3 * C].rearrange("(k p) -> p k", p=P))
    bp_bf = const.tile([1, C], BF16)
    nc.gpsimd.dma_start(out=bp_bf, in_=bp_d.rearrange("(o c) -> o c", o=1))
    # w_proj -> bf16 [128, k, 1024] via one casting SWDGE dma
    wproj_bf = persist.tile([P, NK, C], BF16)
    nc.gpsimd.dma_start(out=wproj_bf, in_=wp_d.rearrange("(k p) n -> p k n", p=P))

    # --- phase X: x -> xT[c-part, k, t] (f32r) ---------------------------
    mm_ps = ctx.enter_context(tc.tile_pool(name="mm_ps", bufs=3, space="PSUM"))
    yT_pool = ctx.enter_context(tc.tile_pool(name="yT", bufs=1))
    xT_ctx = ExitStack()
    xT_pool = xT_ctx.enter_context(tc.tile_pool(name="xT_pool", bufs=1))
    xT = xT_pool.tile([P, NK, T], F32R, name="xT")
    wv_ctx = ExitStack()
    wv_pool = wv_ctx.enter_context(tc.tile_pool(name="wv", bufs=1))
    wvs = []
    with tc.tile_pool(name="xnat", bufs=2) as xnat, \
         tc.tile_pool(name="tp_ps", bufs=4, space="PSUM") as tp_ps:
        for tch in range(NT):
            xn = xnat.tile([P, C], F32R, tag="xn", name=f"xn_{tch}")
            nc.sync.dma_start(out=xn, in_=x_d[tch * P : (tch + 1) * P, :])
            for k in range(NK):
                ps = tp_ps.tile([P, P], F32R, tag="tp", name=f"tp_{tch}_{k}")
                nc.tensor.transpose(ps, xn[:, k * P : (k + 1) * P], ident)
                nc.vector.tensor_copy(xT[:, k, tch * P : (tch + 1) * P], ps)
        # wv loads (f32r direct, 4KB lines)
        for k in range(NK):
            wv = wv_pool.tile([P, C], F32R, tag=f"wv_{k}", name=f"wv_{k}")
            nc.sync.dma_start(out=wv, in_=wa_d[k * P : (k + 1) * P, 2 * C : 3 * C])
            wvs.append(wv)

    # --- phase V: v natural -> vaug[tk-part, tch, h, 65] bf16 ------------
    # column 64 of each head block is 1.0 (fused softmax denominator)
    vaug = persist.tile([P, NT, H, D + 1], BF16)
    nc.gpsimd.memset(vaug[:, :, :, D], 1.0)
    for tch in range(NT):
        pss = [mm_ps.tile([P, TQG], F32, tag="mm", name=f"vps_{tch}_{n}")
               for n in range(2)]
        for k in range(NK):
            for n in range(2):
                nc.tensor.matmul(
                    pss[n], lhsT=xT[:, k, tch * P : (tch + 1) * P],
                    rhs=wvs[k][:, n * TQG : (n + 1) * TQG],
                    start=(k == 0), stop=(k == NK - 1),
                )
        for n in range(2):
            nc.vector.tensor_copy(
                vaug[:, tch, n * 8 : (n + 1) * 8, 0:D],
                pss[n].rearrange("p (h d) -> p h d", h=8),
            )
    wv_ctx.close()

    # --- b_eff = b_v @ w_proj + b_proj  (bf16 row) -----------------------
    b_eff = const.tile([1, C], BF16)
    with tc.tile_pool(name="beff_ps", bufs=2, space="PSUM") as beff_ps:
        bps = [beff_ps.tile([1, TQG], F32, tag="bps", name=f"bps_{n}")
               for n in range(2)]
        for k in range(NK):
            for n in range(2):
                nc.tensor.matmul(
                    bps[n], lhsT=bv_bf[:, k : k + 1],
                    rhs=wproj_bf[:, k, n * TQG : (n + 1) * TQG],
                    start=(k == 0), stop=(k == NK - 1),
                )
        for n in range(2):
            nc.vector.tensor_tensor(
                out=b_eff[0:1, n * TQG : (n + 1) * TQG], in0=bps[n],
                in1=bp_bf[0:1, n * TQG : (n + 1) * TQG], op=mybir.AluOpType.add,
            )

    # --- phase QK/ATTN: rounds -------------------------------------------
    qkT_ctx = ExitStack()
    qkT_pool = qkT_ctx.enter_context(tc.tile_pool(name="qkT", bufs=1))
    wa_pool = qkT_ctx.enter_context(tc.tile_pool(name="wa", bufs=4))
    sps_pool = qkT_ctx.enter_context(tc.tile_pool(name="sps", bufs=2, space="PSUM"))
    tps_pool = qkT_ctx.enter_context(tc.tile_pool(name="tps", bufs=1, space="PSUM"))
    yps_pool = qkT_ctx.enter_context(tc.tile_pool(name="yps", bufs=2, space="PSUM"))
    e_pool = qkT_ctx.enter_context(tc.tile_pool(name="epool", bufs=20))
    nrm_pool = qkT_ctx.enter_context(tc.tile_pool(name="nrm", bufs=16))

    qk_tiles = {}
    wa_tiles = {}
    yT_tiles = {}

    def emit_wa_dma(m):
        wa_m = wa_pool.tile([P, NK, P], F32R, tag="wa", name=f"wa_{m}")
        nc.sync.dma_start(
            out=wa_m,
            in_=wa_d[:, m * P : (m + 1) * P].rearrange("(k p) m -> p k m", p=P),
        )
        wa_tiles[m] = wa_m

    def qk_matmul_list(m):
        """Return the 16 closures emitting chunk m's matmuls + drain."""
        qk = qkT_pool.tile([P, T], BF16, tag=f"qk_{m}", name=f"qkT_{m}")
        qk_tiles[m] = qk
        pss = [mm_ps.tile([P, TQG], F32, tag="mm", name=f"qkps_{m}_{g}")
               for g in range(NG)]
        ops = []
        for k in range(NK):
            for g in range(NG):
                def op(k=k, g=g):
                    nc.tensor.matmul(
                        pss[g], lhsT=wa_tiles[m][:, k, :],
                        rhs=xT[:, k, g * TQG : (g + 1) * TQG],
                        start=(k == 0), stop=(k == NK - 1),
                    )
                    if k == NK - 1:
                        nc.vector.tensor_scalar_add(
                            qk_tiles[m][:, g * TQG : (g + 1) * TQG], pss[g],
                            ba_sb[:, m : m + 1],
                        )
                ops.append(op)
        return ops

    def emit_attn_round(hp, g, qk_ops):
        """Scores+exp+mask for (hp, g), interleaved with qk chunk matmuls,
        then AV, normalization, and the yT dma-transpose."""
        q_tile, k_tile = qk_tiles[hp], qk_tiles[NK + hp]
        if g == 0:
            yT_hp = yT_pool.tile([P, T], BF16, tag=f"yT_{hp}", name=f"yT_{hp}")
            yT_tiles[hp] = yT_hp
        yT_hp = yT_tiles[hp]
        n_i = 4 * g + 4
        e_tiles = {}
        qi = 0
        for i in range(n_i):
            j = i - 4 * g
            lo = max(j, 0) * P
            for head, pl in (("A", 0), ("B", 64)):
                sp = sps_pool.tile([P, TQG], F32, tag="sps",
                                   name=f"sps_{hp}_{g}_{i}_{head}")
                nc.tensor.matmul(
                    sp[:, lo:TQG],
                    lhsT=k_tile[pl : pl + 64, i * P : (i + 1) * P],
                    rhs=q_tile[pl : pl + 64, g * TQG + lo : (g + 1) * TQG],
                    tile_position=(pl, 0),
                )
                e = e_pool.tile([P, TQG], BF16, tag="e",
                                name=f"e_{hp}_{g}_{i}_{head}")
                nc.scalar.activation(e[:, lo:TQG], sp[:, lo:TQG], AF.Exp,
                                     scale=float(SCALE))
                if j >= 0:
                    nc.gpsimd.affine_select(
                        out=e[:, lo : lo + P], in_=e[:, lo : lo + P],
                        compare_op=mybir.AluOpType.is_ge, fill=0.0,
                        base=0, pattern=[[1, P]], channel_multiplier=-1,
                    )
                e_tiles[(i, head)] = e
            # interleave a few qk-chunk matmuls to hide exp latency
            take = (2 * len(qk_ops) * (i + 1)) // (2 * n_i) - qi
            for _ in range(take):
                qk_ops[qi]()
                qi += 1
        while qi < len(qk_ops):
            qk_ops[qi]()
            qi += 1
        # AV: packed psum [128 tq, 4, 65] per head; first matmul in the bank
        # uses start=True (zeroes the whole bank), everything else accumulates
        yps = {}
        for head, pl in (("A", 0), ("B", 64)):
            yps[head] = yps_pool.tile([P, 4, D + 1], F32, tag="yps",
                                      name=f"yps_{hp}_{g}_{head}")
        for i in range(n_i):
            j = i - 4 * g
            for head, pl in (("A", 0), ("B", 64)):
                h = 2 * hp + (0 if head == "A" else 1)
                e = e_tiles[(i, head)]
                for tcl in range(max(j, 0), 4):
                    nc.tensor.matmul(
                        yps[head][:, tcl, :],
                        lhsT=e[:, tcl * P : (tcl + 1) * P],
                        rhs=vaug[:, i, h, :],
                        start=(i == 0 and tcl == max(j, 0)),
                        stop=(j == tcl),
                        skip_group_check=True,
                    )
        if hp == 0 and g == 0:
            dump("e00A", e_tiles[(0, "A")], [P, TQG])
            dump("ypsA", yps["A"].rearrange("p a d -> p (a d)"), [P, 4 * (D + 1)])
        if hp == 0 and g == 1:
            dump("e10A", e_tiles[(0, "A")], [P, TQG])
            dump("e17A", e_tiles[(7, "A")], [P, TQG])
            dump("yps1A", yps["A"].rearrange("p a d -> p (a d)"), [P, 4 * (D + 1)])
        if hp == 3 and g == 0:
            dump("qk3", qk_tiles[3], [P, T])
            dump("qk11", qk_tiles[NK + 3], [P, T])
            dump("e30A", e_tiles[(0, "A")], [P, TQG])
            dump("yps3A", yps["A"].rearrange("p a d -> p (a d)"), [P, 4 * (D + 1)])
        # normalize into y_blk [tq-part, 4, 128] (cols: headA 0:64, headB 64:128)
        y_blk = nrm_pool.tile([P, 4, P], BF16, tag="yblk", name=f"yblk_{hp}_{g}")
        for head, off in (("A", 0), ("B", 64)):
            recip = nrm_pool.tile([P, 4], F32, tag="recip",
                                  name=f"recip_{hp}_{g}_{head}")
            nc.vector.reciprocal(recip, yps[head][:, :, D])
            nc.vector.tensor_tensor(
                out=y_blk[:, :, off : off + D], in0=yps[head][:, :, 0:D],
                in1=recip[:, :, None].broadcast_to((P, 4, D)),
                op=mybir.AluOpType.mult,
            )
        if hp == 0 and g == 0:
            dump("yblk00", y_blk.rearrange("p a b -> p (a b)"), [P, 4 * P])
        if hp == 0 and g == 1:
            dump("yblk01", y_blk.rearrange("p a b -> p (a b)"), [P, 4 * P])
        if hp == 3 and g == 0:
            dump("yblk30", y_blk.rearrange("p a b -> p (a b)"), [P, 4 * P])
        # yT[hp][:, g*512:(g+1)*512] <- blockwise PE transpose of y_blk
        for tcl in range(4):
            tp = tps_pool.tile([P, P], BF16, tag="ytp", name=f"ytp_{hp}_{g}_{tcl}")
            nc.tensor.transpose(tp, y_blk[:, tcl, :], ident_bf)
            nc.vector.tensor_copy(
                yT_hp[:, g * TQG + tcl * P : g * TQG + (tcl + 1) * P], tp)

    # bootstrap: weight dmas + first two qk chunks
    chunk_order = [0, NK] + [m for pair in
                             [(i, NK + i) for i in range(1, NK)] for m in pair]
    dma_cursor = 0
    for _ in range(4):  # prefetch 0, 8, 1, 9
        emit_wa_dma(chunk_order[dma_cursor])
        dma_cursor += 1
    for ops in (qk_matmul_list(0), qk_matmul_list(NK)):
        for op in ops:
            op()

    dump("qk0", qk_tiles[0], [P, T])
    dump("qk8", qk_tiles[NK], [P, T])
    dump("vaug", vaug.rearrange("p a h d -> p (a h d)"), [P, NT * H * (D + 1)])

    rounds = [(hp, g) for hp in range(HPAIRS) for g in range(NG)]
    for r, (hp, g) in enumerate(rounds):
        if dma_cursor < len(chunk_order):
            emit_wa_dma(chunk_order[dma_cursor])
            dma_cursor += 1
        emit_idx = 2 + r  # chunk_order position whose matmuls run this round
        qk_ops = qk_matmul_list(chunk_order[emit_idx]) if emit_idx < 16 else []
        emit_attn_round(hp, g, qk_ops)

    qkT_ctx.close()
    xT_ctx.close()

    dump("yT0", yT_tiles[0], [P, T])
    dump("wproj01", wproj_bf[:, 0, :], [P, C])
    dump("wproj7", wproj_bf[:, 7, :], [P, C])
    dump("beff", b_eff, [1, C])
    dump("yT3", yT_tiles[3], [P, T])
    dump("yT7", yT_tiles[7], [P, T])

    # --- phase OUT: out = yT^T-contract @ w_proj + b_eff -----------------
    out_sb = ctx.enter_context(tc.tile_pool(name="out_sb", bufs=3))
    for m in range(NT):
        pss = [mm_ps.tile([P, TQG], F32, tag="mm", name=f"pps_{m}_{n}")
               for n in range(2)]
        for hp in range(HPAIRS):
            for n in range(2):
                nc.tensor.matmul(
                    pss[n], lhsT=yT_tiles[hp][:, m * P : (m + 1) * P],
                    rhs=wproj_bf[:, hp, n * TQG : (n + 1) * TQG],
                    start=(hp == 0), stop=False,
                )
        ob = out_sb.tile([P, C], F32, tag="ob", name=f"ob_{m}")
        for n in range(2):
            nc.tensor.matmul(
                pss[n], lhsT=ones_bf[0:1, 0:P],
                rhs=b_eff[0:1, n * TQG : (n + 1) * TQG],
                start=False, stop=True,
            )
            if n == 0:
                nc.vector.tensor_copy(ob[:, n * TQG : (n + 1) * TQG], pss[n])
            else:
                nc.scalar.copy(ob[:, n * TQG : (n + 1) * TQG], pss[n])
        nc.sync.dma_start(out=out_d[m * P : (m + 1) * P, :], in_=ob)


DBG_SHAPES = {
    "wproj01": [P, C], "wproj7": [P, C], "beff": [1, C],
    "yT0": [P, T], "yT3": [P, T], "yT7": [P, T],
}


def build_nc_dbg(n_cores=1):
    nc = bass.Bass("TRN2", target_bir_lowering=False, debug=False, num_devices=n_cores)
    x_d = nc.dram_tensor("x", [T, C], F32R, kind="ExternalInput").ap()
    wa_d = nc.dram_tensor("w_attn", [C, C3], F32R, kind="ExternalInput").ap()
    ba_d = nc.dram_tensor("b_attn", [C3], F32, kind="ExternalInput").ap()
    wp_d = nc.dram_tensor("w_proj", [C, C], F32, kind="ExternalInput").ap()
    bp_d = nc.dram_tensor("b_proj", [C], F32, kind="ExternalInput").ap()
    out_d = nc.dram_tensor("out", [T, C], F32, kind="ExternalOutput").ap()
    dbg = {k: nc.dram_tensor(f"dbg_{k}", v, F32, kind="ExternalOutput").ap()
           for k, v in DBG_SHAPES.items()}
    with tile.TileContext(nc) as tc:
        with nc.allow_low_precision(reason="f32r/bf16 matmul inputs are intentional"):
            with ExitStack() as ctx:
                _emit_kernel(nc, tc, ctx, x_d, wa_d, ba_d, wp_d, bp_d, out_d, dbg=dbg)
    _legalize_waits(nc)
    return nc


def build_nc(n_cores=N_CORES, reps=1):
    nc = bass.Bass("TRN2", target_bir_lowering=False, debug=False, num_devices=n_cores)
    x_d = nc.dram_tensor("x", [T, C], F32R, kind="ExternalInput").ap()
    wa_d = nc.dram_tensor("w_attn", [C, C3], F32R, kind="ExternalInput").ap()
    ba_d = nc.dram_tensor("b_attn", [C3], F32, kind="ExternalInput").ap()
    wp_d = nc.dram_tensor("w_proj", [C, C], F32, kind="ExternalInput").ap()
    bp_d = nc.dram_tensor("b_proj", [C], F32, kind="ExternalInput").ap()
    out_d = nc.dram_tensor("out", [T, C], F32, kind="ExternalOutput").ap()
    with tile.TileContext(nc) as tc:
        with nc.allow_low_precision(reason="f32r/bf16 matmul inputs are intentional"):
            for _ in range(reps):
                with ExitStack() as ctx:
                    _emit_kernel(nc, tc, ctx, x_d, wa_d, ba_d, wp_d, bp_d, out_d)
    _legalize_waits(nc)
    return nc


_NC_CACHE = {}


def _get_nc(n_cores=N_CORES):
    if n_cores not in _NC_CACHE:
        _NC_CACHE[n_cores] = build_nc(n_cores)
    return _NC_CACHE[n_cores]


def kernel(x, attn_mask, w_attn, b_attn, w_proj, b_proj):
    """Full inputs in, full output out. attn_mask is causal (hardcoded)."""
    x = np.ascontiguousarray(np.asarray(x, dtype=np.float32))
    w_attn = np.ascontiguousarray(np.asarray(w_attn, dtype=np.float32))
    b_attn = np.ascontiguousarray(np.asarray(b_attn, dtype=np.float32))
    w_proj = np.ascontiguousarray(np.asarray(w_proj, dtype=np.float32))
    b_proj = np.ascontiguousarray(np.asarray(b_proj, dtype=np.float32))
    B = x.shape[0]
    assert B == N_CORES and x.shape == (B, T, C)

    nc = _get_nc(N_CORES)
    in_maps = [
        {"x": x[b], "w_attn": w_attn, "b_attn": b_attn,
         "w_proj": w_proj, "b_proj": b_proj}
        for b in range(B)
    ]
    res = run_bass_kernel_spmd(nc, in_maps, core_ids=list(range(N_CORES)))
    return np.stack([res.results[b]["out"] for b in range(B)], axis=0)
